# revision 1
# baseline (speedup 1.0000x reference)
"""Causal self-attention Trainium2 kernel.

Problem: x[4,2048,1024] -> qkv proj -> 16-head causal attention -> out proj.

Sharding (8 cores): core = 2*batch + head_half. Each core handles one batch
(T=2048 tokens) and 8 of the 16 heads:
  - computes q^T,k^T (feature-major) and v (token-major) for its heads
  - S^T[j,i] = k^T.T-free attention scores, exp (no max-subtraction: logits
    are O(5) std-normal so exp is safe in fp32), causal mask, P^T @ v via
    an appended ones-column that yields the softmax row-sums for free
  - normalizes y, then computes the partial out-projection for its 512
    feature rows of w_out.
Host sums the two half-head partials per batch and adds biases. b_v is folded
in on the host via b_v @ w_out (exact since softmax rows sum to 1); b_out is
added on the host too.

All matmuls run as float32r (full-rate PE mode, fp32 storage).
"""

import numpy as np
from contextlib import ExitStack

import concourse.bass as bass
from concourse import bacc, mybir, tile
from concourse.bass_utils import run_bass_kernel_spmd

F32 = mybir.dt.float32
F32R = mybir.dt.float32r
AF = mybir.ActivationFunctionType

B = 4
T = 2048
C = 1024
H = 16
D = 64
SCALE = 1.0 / np.sqrt(D)

HL = 8            # heads per core
F = HL * D        # 512 local feature columns
NCC = C // 128    # 8 contraction chunks
NFT = F // 128    # 4 feature tiles (2 heads each)
NTT = T // 128    # 16 token tiles
NTB = T // 512    # 4 token blocks
DA = D + 1        # head dim + ones column


def build_nc():
    nc = bacc.Bacc()

    x = nc.declare_dram_parameter("x", [T, C], F32R, isOutput=False)
    wq = nc.declare_dram_parameter("wq", [C, F], F32R, isOutput=False)
    wk = nc.declare_dram_parameter("wk", [C, F], F32R, isOutput=False)
    wv = nc.declare_dram_parameter("wv", [C, F], F32R, isOutput=False)
    wout = nc.declare_dram_parameter("wout", [F, C], F32R, isOutput=False)
    bq = nc.declare_dram_parameter("bq", [128, NFT], F32, isOutput=False)
    bk = nc.declare_dram_parameter("bk", [128, NFT], F32, isOutput=False)
    ident = nc.declare_dram_parameter("ident", [128, 128], F32R, isOutput=False)
    maskp0 = nc.declare_dram_parameter("maskp0", [128, 1024], F32R, isOutput=False)
    maskp1 = nc.declare_dram_parameter("maskp1", [128, 1024], F32R, isOutput=False)
    outp = nc.declare_dram_parameter("out", [T, C], F32, isOutput=True)

    with tile.TileContext(nc) as tc, ExitStack() as top:
        const = top.enter_context(tc.tile_pool(name="const", bufs=1))
        dram = top.enter_context(tc.tile_pool(name="dram", bufs=1, space="DRAM"))
        yt_dram = [
            dram.tile([F, 512], F32R, name=f"yt_dram{g}", tag=f"yt{g}")
            for g in range(NTB)
        ]

        # v with an appended ones column per head: [token, 8*(64+1)]
        vaug = top.enter_context(tc.tile_pool(name="vaug", bufs=1))
        v_ch = [
            vaug.tile([128, HL * DA], F32R, name=f"v{t}", tag=f"v{t}")
            for t in range(NTT)
        ]

        # x^T, contraction dim on partitions: 8 chunks of [128, T]
        xt_pool = top.enter_context(tc.tile_pool(name="xt", bufs=1))
        xt = [
            xt_pool.tile([128, T], F32R, name=f"xt{c}", tag=f"xt{c}")
            for c in range(NCC)
        ]

        # small attention epilogue tiles
        yts = top.enter_context(tc.tile_pool(name="yts", bufs=2))

        # out-projection weights: loaded once during startup
        woutp = top.enter_context(tc.tile_pool(name="woutp", bufs=1))

        # x staging, released after phase 1 (must be stack-top at release)
        xq = tc.alloc_tile_pool(name="xq", bufs=3)

        # DMA emission order: ident + first x tile first, so the PE can
        # start transposing while the bulk constant loads stream behind
        ident_sb = const.tile([128, 128], F32R, name="ident_sb")
        nc.sync.dma_start(out=ident_sb, in_=ident[:, :])
        xloads = [None] * NTT
        xloads[0] = xq.tile([128, C], F32R, name="xload0", tag="xload")
        nc.scalar.dma_start(out=xloads[0][:, 0:512], in_=x[0:128, 0:512])
        nc.sync.dma_start(out=xloads[0][:, 512:1024], in_=x[0:128, 512:1024])
        mask_sb = []
        for i, msrc in enumerate((maskp0, maskp1)):
            mt = const.tile([128, 1024], F32R, name=f"mask{i}", tag=f"mask{i}")
            nc.sync.dma_start(out=mt, in_=msrc[:, :])
            mask_sb.append(mt)
        bq_sb = const.tile([128, NFT], F32, name="bq_sb")
        nc.sync.dma_start(out=bq_sb, in_=bq[:, :])
        bk_sb = const.tile([128, NFT], F32, name="bk_sb")
        nc.sync.dma_start(out=bk_sb, in_=bk[:, :])
        ones_sb = const.tile([128, HL], F32, name="ones_sb")
        nc.gpsimd.memset(ones_sb, 1.0)
        wout_sb = [
            woutp.tile([128, C], F32R, name=f"wout{fc}", tag=f"wout{fc}")
            for fc in range(NFT)
        ]

        # ---- Phase 1: transpose x and project v (fused per token tile) ----
        with tc.tile_pool(name="wvp", bufs=1) as wvp, \
             tc.tile_pool(name="ph1ps", bufs=5, space="PSUM") as ph1ps, \
             tc.tile_pool(name="vps", bufs=3, space="PSUM") as vps:
            wv_sb = []
            for cc in range(NCC):
                wvt = wvp.tile([128, F], F32R, name=f"wv{cc}", tag=f"wv{cc}")
                nc.sync.dma_start(out=wvt, in_=wv[128 * cc:128 * (cc + 1), :])
                wv_sb.append(wvt)
            for tt in range(NTT):
                xload = xloads[tt]
                if tt + 1 < NTT:
                    xloads[tt + 1] = xq.tile(
                        [128, C], F32R, name=f"xload{tt + 1}", tag="xload"
                    )
                    nc.scalar.dma_start(
                        out=xloads[tt + 1],
                        in_=x[128 * (tt + 1):128 * (tt + 2), :],
                    )
                for cc in range(NCC):
                    xt_ps = ph1ps.tile([128, 128], F32R, name="xt_ps", tag="xt_ps")
                    nc.tensor.transpose(
                        xt_ps, xload[:, 128 * cc:128 * (cc + 1)], ident_sb
                    )
                    nc.vector.tensor_copy(xt[cc][:, 128 * tt:128 * (tt + 1)], xt_ps)
                v_ps = vps.tile([128, F], F32, name="v_ps", tag="v_ps")
                for cc in range(NCC):
                    nc.tensor.matmul(
                        v_ps,
                        lhsT=xt[cc][:, 128 * tt:128 * (tt + 1)],
                        rhs=wv_sb[cc],
                        start=(cc == 0),
                        stop=(cc == NCC - 1),
                    )
                vv = v_ch[tt].rearrange("p (h c) -> p h c", c=DA)
                nc.vector.tensor_copy(
                    vv[:, :, 0:D], v_ps.rearrange("p (h c) -> p h c", c=D)
                )
                nc.vector.tensor_copy(
                    vv[:, :, D:DA], ones_sb.rearrange("p (h o) -> p h o", o=1)
                )
        xq.release()

        # ---- Phase 2: per feature-tile: q/k projection + attention ----
        with tc.tile_pool(name="ph2", bufs=1) as ph2, \
             tc.tile_pool(name="wqkp", bufs=2) as wqkp, \
             tc.tile_pool(name="e2p", bufs=1) as e2p, \
             tc.tile_pool(name="ph2ps", bufs=2, space="PSUM") as ph2ps:
            for fc in range(NFT):
                nc.sync.dma_start(
                    out=wout_sb[fc], in_=wout[128 * fc:128 * (fc + 1), :]
                )
            pend = None

            def finish_block(b, yA, yB, hA, hB, pv_pair):
                # the two deferred score pairs' PV close both accumulation
                # groups, then each head's output block is normalized and
                # staged to DRAM
                if 2 * b + 2 >= 2:
                    pv_pair(2 * b, last=False)
                pv_pair(2 * b + 1, last=True)
                for y_ps, h in ((yA, hA), (yB, hB)):
                    recip_t = yts.tile([1, 512], F32, name="recip", tag="recip")
                    nc.vector.reciprocal(recip_t, y_ps[D:DA, :])
                    rb_t = yts.tile([64, 512], F32, name="rb", tag="rb")
                    nc.gpsimd.partition_broadcast(rb_t, recip_t[0:1, :])
                    yt_t = yts.tile([64, 512], F32R, name="yt", tag="yt")
                    nc.vector.tensor_mul(yt_t, y_ps[0:D, :], rb_t)
                    nc.sync.dma_start(
                        out=yt_dram[b][64 * h:64 * (h + 1), :],
                        in_=yt_t,
                    )

            def load_wqk(ft):
                wqf, wkf = [], []
                for cc in range(NCC):
                    wq_t = wqkp.tile(
                        [128, 128], F32R, name=f"wqf{cc}", tag=f"wqf{cc}"
                    )
                    nc.sync.dma_start(
                        out=wq_t,
                        in_=wq[128 * cc:128 * (cc + 1), 128 * ft:128 * (ft + 1)],
                    )
                    wqf.append(wq_t)
                    wk_t = wqkp.tile(
                        [128, 128], F32R, name=f"wkf{cc}", tag=f"wkf{cc}"
                    )
                    nc.sync.dma_start(
                        out=wk_t,
                        in_=wk[128 * cc:128 * (cc + 1), 128 * ft:128 * (ft + 1)],
                    )
                    wkf.append(wk_t)
                return wqf, wkf

            def emit_proj(ft, wqk):
                qft = ph2.tile([128, T], F32R, name="qft", tag="qft", bufs=1)
                kft = ph2.tile([128, T], F32R, name="kft", tag="kft", bufs=1)
                wqf, wkf = wqk
                for tb in range(NTB):
                    q_ps = ph2ps.tile(
                        [128, 512], F32, name="q_ps", tag="bank1", bufs=4
                    )
                    for cc in range(NCC):
                        nc.tensor.matmul(
                            q_ps,
                            lhsT=wqf[cc],
                            rhs=xt[cc][:, 512 * tb:512 * (tb + 1)],
                            start=(cc == 0),
                            stop=(cc == NCC - 1),
                        )
                    nc.vector.tensor_scalar_add(
                        qft[:, 512 * tb:512 * (tb + 1)], q_ps, bq_sb[:, ft:ft + 1]
                    )
                    k_ps = ph2ps.tile(
                        [128, 512], F32, name="k_ps", tag="bank1", bufs=4
                    )
                    for cc in range(NCC):
                        nc.tensor.matmul(
                            k_ps,
                            lhsT=wkf[cc],
                            rhs=xt[cc][:, 512 * tb:512 * (tb + 1)],
                            start=(cc == 0),
                            stop=(cc == NCC - 1),
                        )
                    nc.vector.tensor_scalar_add(
                        kft[:, 512 * tb:512 * (tb + 1)], k_ps, bk_sb[:, ft:ft + 1]
                    )
                return qft, kft

            wqk_next = load_wqk(0)
            for ft in range(NFT):
                qft, kft = emit_proj(ft, wqk_next)
                if ft + 1 < NFT:
                    wqk_next = load_wqk(ft + 1)

                # Both heads of this feature tile processed together: their
                # K=64 S^T matmuls sit in disjoint PE row groups (partition
                # bases 0 and 64), so adjacent emission lets the hardware
                # run each A/B pair concurrently. PV consumes score pairs
                # one pair behind the exp stream; the final pair's PV and
                # the normalization are deferred into the next block so the
                # PE never blocks on this block's activations.
                hA = 2 * ft
                hB = 2 * ft + 1
                for b in range(NTB):
                    npair = 2 * b + 2
                    yA = ph2ps.tile([DA, 512], F32, name="yA", tag="bank1", bufs=4)
                    yB = ph2ps.tile([DA, 512], F32, name="yB", tag="bank1", bufs=4)
                    eA, eB = {}, {}

                    def pv_pair(tp, last, _b=b, _yA=yA, _yB=yB, _eA=eA, _eB=eB,
                                _hA=hA, _hB=hB):
                        # default-arg binding: this closure is also called
                        # from the NEXT block via `pend`, after these names
                        # have been rebound
                        nt = 4 * _b + 4
                        for t in (2 * tp, 2 * tp + 1):
                            for y_ps, h, e in (
                                (_yA, _hA, _eA), (_yB, _hB, _eB)
                            ):
                                nc.tensor.matmul(
                                    y_ps,
                                    lhsT=v_ch[t][:, DA * h:DA * (h + 1)],
                                    rhs=e[tp][:, 512 * (t % 2):512 * (t % 2 + 1)],
                                    start=(t == 0),
                                    stop=(t == nt - 1) and last,
                                    skip_group_check=True,
                                )

                    for tp in range(npair):
                        sA = ph2ps.tile([128, 1024], F32, name="sA", tag="sA", bufs=1)
                        sB = ph2ps.tile([128, 1024], F32, name="sB", tag="sB", bufs=1)
                        for half in range(2):
                            t = 2 * tp + half
                            for s_ps, po in ((sA, 0), (sB, 64)):
                                nc.tensor.matmul(
                                    s_ps[:, 512 * half:512 * (half + 1)],
                                    lhsT=kft[po:po + 64, 128 * t:128 * (t + 1)],
                                    rhs=qft[po:po + 64, 512 * b:512 * (b + 1)],
                                    start=True,
                                    stop=True,
                                )
                        eA[tp] = e2p.tile(
                            [128, 1024], F32R, name=f"e2a{tp % 4}",
                            tag=f"e2a{tp % 4}",
                        )
                        nc.scalar.activation(eA[tp], sA, AF.Exp, scale=SCALE)
                        eB[tp] = e2p.tile(
                            [128, 1024], F32R, name=f"e2b{tp % 4}",
                            tag=f"e2b{tp % 4}",
                        )
                        nc.scalar.activation(eB[tp], sB, AF.Exp, scale=SCALE)
                        if tp >= npair - 2:
                            mk = mask_sb[tp - (npair - 2)]
                            nc.gpsimd.tensor_mul(eA[tp], eA[tp], mk)
                            nc.gpsimd.tensor_mul(eB[tp], eB[tp], mk)
                        if tp == 0 and pend is not None:
                            finish_block(*pend)
                            pend = None
                        if tp >= 2:
                            pv_pair(tp - 2, last=False)
                    pend = (b, yA, yB, hA, hB, pv_pair)
            finish_block(*pend)

        # ---- Phase 3: partial out-projection ----
        with tc.tile_pool(name="ph3", bufs=2) as ph3, \
             tc.tile_pool(name="ph3ps", bufs=4, space="PSUM") as ph3ps:
            def load_ytl(g):
                # one [128, 512] load per feature chunk covers 4 token tiles
                # with 2KB-contiguous rows (good DMA descriptor efficiency)
                tiles = []
                for fc in range(NFT):
                    yl = ph3.tile(
                        [128, 512], F32R, name=f"ytl{fc}", tag=f"ytl{fc}"
                    )
                    nc.sync.dma_start(
                        out=yl,
                        in_=yt_dram[g][128 * fc:128 * (fc + 1), :],
                    )
                    tiles.append(yl)
                return tiles

            ytl = load_ytl(0)
            for g in range(NTB):
                ytl_next = load_ytl(g + 1) if g + 1 < NTB else None
                for ti in range(4):
                    tt = 4 * g + ti
                    for eb in range(2):
                        o_ps = ph3ps.tile([128, 512], F32, name="o_ps", tag="o_ps")
                        for fc in range(NFT):
                            nc.tensor.matmul(
                                o_ps,
                                lhsT=ytl[fc][:, 128 * ti:128 * (ti + 1)],
                                rhs=wout_sb[fc][:, 512 * eb:512 * (eb + 1)],
                                start=(fc == 0),
                                stop=(fc == NFT - 1),
                            )
                        osb = ph3.tile(
                            [128, 512], F32, name="osb", tag="osb", bufs=3
                        )
                        nc.vector.tensor_copy(osb, o_ps)
                        nc.scalar.dma_start(
                            out=outp[
                                128 * tt:128 * (tt + 1), 512 * eb:512 * (eb + 1)
                            ],
                            in_=osb,
                        )
                ytl = ytl_next

    nc.finalize()
    return nc


def _host_constants():
    ident = np.eye(128, dtype=np.float32)
    # mask_k[jj, ii] = 1 if (128k + jj) <= ii  (keep j <= i within diagonal blk)
    masks = []
    jj = np.arange(128)[:, None]
    ii = np.arange(512)[None, :]
    for k in range(4):
        masks.append(((128 * k + jj) <= ii).astype(np.float32))
    maskp0 = np.concatenate([masks[0], masks[1]], axis=1)
    maskp1 = np.concatenate([masks[2], masks[3]], axis=1)
    return ident, np.ascontiguousarray(maskp0), np.ascontiguousarray(maskp1)


def _in_maps(x, w_qkv, b_qkv, w_out):
    ident, maskp0, maskp1 = _host_constants()
    maps = []
    for core in range(8):
        b, g = core // 2, core % 2
        cols = slice(512 * g, 512 * (g + 1))
        wq = np.ascontiguousarray(w_qkv[:, 0 * C:1 * C][:, cols])
        wk = np.ascontiguousarray(w_qkv[:, 1 * C:2 * C][:, cols])
        wv = np.ascontiguousarray(w_qkv[:, 2 * C:3 * C][:, cols])
        bq = np.ascontiguousarray(
            b_qkv[0 * C:1 * C][cols].reshape(NFT, 128).T
        )
        bk = np.ascontiguousarray(
            b_qkv[1 * C:2 * C][cols].reshape(NFT, 128).T
        )
        wo = np.ascontiguousarray(w_out[cols, :])
        maps.append(
            {
                "x": np.ascontiguousarray(x[b]),
                "wq": wq,
                "wk": wk,
                "wv": wv,
                "wout": wo,
                "bq": bq,
                "bk": bk,
                "ident": ident,
                "maskp0": maskp0,
                "maskp1": maskp1,
            }
        )
    return maps


_NC_CACHE = {}


def _get_nc():
    if "nc" not in _NC_CACHE:
        _NC_CACHE["nc"] = build_nc()
    return _NC_CACHE["nc"]


def run(inputs, trace=False, **spmd_kwargs):
    """Returns (output, BassKernelResults)."""
    x = np.asarray(inputs["x"], dtype=np.float32)
    w_qkv = np.asarray(inputs["w_qkv"], dtype=np.float32)
    b_qkv = np.asarray(inputs["b_qkv"], dtype=np.float32)
    w_out = np.asarray(inputs["w_out"], dtype=np.float32)
    b_out = np.asarray(inputs["b_out"], dtype=np.float32)

    nc = _get_nc()
    maps = _in_maps(x, w_qkv, b_qkv, w_out)
    res = run_bass_kernel_spmd(
        nc, maps, list(range(8)), trace=trace, **spmd_kwargs
    )
    out = np.empty((B, T, C), dtype=np.float32)
    for b in range(B):
        out[b] = res.results[2 * b]["out"] + res.results[2 * b + 1]["out"]
    # softmax rows sum to 1, so v-bias passes through attention unchanged:
    # its contribution to the output is b_v @ w_out, added once on the host.
    bias = b_out + b_qkv[2 * C:3 * C] @ w_out
    out += bias[None, None, :]
    return out, res


def kernel(**inputs):
    out, _ = run(inputs, trace=False)
    return out



# revision 14
# speedup vs baseline: 1.2440x; 1.2440x over previous
"""Causal self-attention Trainium2 kernel.

Problem: x[4,2048,1024] -> qkv proj -> 16-head causal attention -> out proj.

Sharding (8 cores): core = 2*batch + head_half. Each core handles one batch
(T=2048 tokens) and 8 of the 16 heads. Host sums the two half-head partials
per batch and adds biases (b_v folded via b_v @ w_out; exact since softmax
rows sum to 1).

Core-side structure (all matmul operands bf16, fp32 PSUM accumulation):
  - host supplies x^T (feature-major) so no on-chip x transpose is needed
  - q^T,k^T projections per head-pair (feature-major), v token-major with an
    appended ones column per head (gives softmax row-sums for free)
  - scores S^T[key, query] per 128-key tile (K=64 per head), causally
    F-trimmed on the 4 diagonal tiles of each 512-query block
  - exp on the scalar engine into bf16; triangular masks (gpsimd) only on
    the 128x128 diagonal blocks; above-diagonal tiles are simply skipped
  - PV transposed: y[128q, 65] psum accumulators per (head, q-slice), lhsT =
    exp-score tile, rhs = v_aug; 65-row matmuls
  - normalization folded into the PSUM->SBUF copy (per-partition reciprocal
    scalars), y^T via PE transposes, out-projection straight from SBUF
  - projection / out-projection / transpose work is interleaved into the
    attention instruction stream as PE filler so the PE never waits on exp
"""

import math
import numpy as np
from contextlib import ExitStack

import concourse.bass as bass
from concourse import bacc, mybir, tile
from concourse.bass_utils import run_bass_kernel_spmd

F32 = mybir.dt.float32
BF16 = mybir.dt.bfloat16
AF = mybir.ActivationFunctionType

B = 4
T = 2048
C = 1024
H = 16
D = 64
SCALE = 1.0 / np.sqrt(D)

HL = 8            # heads per core
F = HL * D        # 512 local feature columns
NCC = C // 128    # 8 contraction chunks
NHP = HL // 2     # 4 head pairs (= feature tiles of 128)
NTT = T // 128    # 16 token tiles
NTB = T // 512    # 4 token blocks
DA = D + 1        # head dim + ones column


_DEBUG = False


def build_nc():
    nc = bacc.Bacc()

    xt_d = nc.declare_dram_parameter("xt", [C, T], BF16, isOutput=False)
    wq_d = nc.declare_dram_parameter("wq", [C, F], BF16, isOutput=False)
    wk_d = nc.declare_dram_parameter("wk", [C, F], BF16, isOutput=False)
    wv_d = nc.declare_dram_parameter("wv", [C, F], BF16, isOutput=False)
    wout_d = nc.declare_dram_parameter("wout", [F, C], BF16, isOutput=False)
    bq_d = nc.declare_dram_parameter("bq", [128, NHP], F32, isOutput=False)
    bk_d = nc.declare_dram_parameter("bk", [128, NHP], F32, isOutput=False)
    tri_d = nc.declare_dram_parameter("tri", [128, 128], BF16, isOutput=False)
    ident_d = nc.declare_dram_parameter("ident", [128, 128], BF16, isOutput=False)
    out_d = nc.declare_dram_parameter("out", [T, C], F32, isOutput=True)
    if _DEBUG:
        dbg_qft = nc.declare_dram_parameter("dbg_qft", [128, T], BF16, isOutput=True)
        dbg_kft = nc.declare_dram_parameter("dbg_kft", [128, T], BF16, isOutput=True)
        dbg_v = nc.declare_dram_parameter("dbg_v", [128, HL * DA], BF16, isOutput=True)
        dbg_e = nc.declare_dram_parameter("dbg_e", [128, 1024], BF16, isOutput=True)
        dbg_yn = nc.declare_dram_parameter("dbg_yn", [128, F], BF16, isOutput=True)

    with tile.TileContext(nc) as tc, ExitStack() as top:
        const = top.enter_context(tc.tile_pool(name="const", bufs=1))
        xtp = top.enter_context(tc.tile_pool(name="xtp", bufs=1))
        wp = top.enter_context(tc.tile_pool(name="wp", bufs=1))
        qkp = top.enter_context(tc.tile_pool(name="qkp", bufs=1))
        vp = top.enter_context(tc.tile_pool(name="vp", bufs=1))
        ep = top.enter_context(tc.tile_pool(name="ep", bufs=3))
        yp = top.enter_context(tc.tile_pool(name="yp", bufs=2))
        smallp = top.enter_context(tc.tile_pool(name="smallp", bufs=2))
        osbp = top.enter_context(tc.tile_pool(name="osbp", bufs=3))
        # PSUM: sA(2) + sB(2) + yA(1) + yB(1) + fill(2x1) = 8 banks
        sps = top.enter_context(tc.tile_pool(name="sps", bufs=1, space="PSUM"))
        yps = top.enter_context(tc.tile_pool(name="yps", bufs=1, space="PSUM"))
        fps = top.enter_context(tc.tile_pool(name="fps", bufs=2, space="PSUM"))

        # ---- persistent SBUF tiles ----
        xt = [xtp.tile([128, T], BF16, name=f"xt{cc}", tag=f"xt{cc}")
              for cc in range(NCC)]
        wq_sb = [wp.tile([128, F], BF16, name=f"wq{cc}", tag=f"wq{cc}")
                 for cc in range(NCC)]
        wk_sb = [wp.tile([128, F], BF16, name=f"wk{cc}", tag=f"wk{cc}")
                 for cc in range(NCC)]
        wv_sb = [wp.tile([128, F], BF16, name=f"wv{cc}", tag=f"wv{cc}")
                 for cc in range(NCC)]
        wout_sb = [wp.tile([128, C], BF16, name=f"wout{fc}", tag=f"wout{fc}")
                   for fc in range(NHP)]
        qft = [qkp.tile([128, T], BF16, name=f"qft{hp}", tag=f"qft{hp}")
               for hp in range(NHP)]
        kft = [qkp.tile([128, T], BF16, name=f"kft{hp}", tag=f"kft{hp}")
               for hp in range(NHP)]
        v_aug = [vp.tile([128, HL * DA], BF16, name=f"v{t}", tag=f"v{t}")
                 for t in range(NTT)]
        ident_sb = const.tile([128, 128], BF16, name="ident_sb")
        tri_sb = const.tile([128, 128], BF16, name="tri_sb")
        bq_sb = const.tile([128, NHP], F32, name="bq_sb")
        bk_sb = const.tile([128, NHP], F32, name="bk_sb")
        ones_sb = const.tile([128, HL], BF16, name="ones_sb")

        # ---- DMA prologue ----
        # Spread the startup-critical loads (wv + x^T quarter 0) across the
        # SP / Act / Pool queues so the first v-proj matmul can start ~2.5us
        # in; the Act queue is free until the first exp (~25us).
        def xt_dma(eng, cc, tq):
            eng.dma_start(
                out=xt[cc][:, 512 * tq:512 * (tq + 1)],
                in_=xt_d[128 * cc:128 * (cc + 1), 512 * tq:512 * (tq + 1)],
            )

        nc.gpsimd.dma_start(out=bq_sb, in_=bq_d[:, :])
        nc.gpsimd.dma_start(out=bk_sb, in_=bk_d[:, :])
        nc.gpsimd.dma_start(out=tri_sb, in_=tri_d[:, :])
        nc.gpsimd.dma_start(out=ident_sb, in_=ident_d[:, :])
        nc.gpsimd.memset(ones_sb, 1.0)
        for cc in range(4):
            nc.sync.dma_start(out=wv_sb[cc],
                              in_=wv_d[128 * cc:128 * (cc + 1), :])
        for cc in range(4, NCC):
            nc.scalar.dma_start(out=wv_sb[cc],
                                in_=wv_d[128 * cc:128 * (cc + 1), :])
        for cc in range(2):
            xt_dma(nc.sync, cc, 0)
        for cc in range(2, 4):
            xt_dma(nc.scalar, cc, 0)
        for cc in range(4, NCC):
            xt_dma(nc.gpsimd, cc, 0)
        for cc in range(NCC):
            nc.gpsimd.dma_start(out=wq_sb[cc],
                                in_=wq_d[128 * cc:128 * (cc + 1), :])
            nc.gpsimd.dma_start(out=wk_sb[cc],
                                in_=wk_d[128 * cc:128 * (cc + 1), :])
        for fc in range(NHP):
            nc.sync.dma_start(out=wout_sb[fc],
                              in_=wout_d[128 * fc:128 * (fc + 1), :])
        for cc in range(NCC):
            xt_dma(nc.sync, cc, 1)
        for cc in range(NCC):
            xt_dma(nc.gpsimd, cc, 2)
            xt_dma(nc.gpsimd, cc, 3)

        # ---- emission helpers ----
        def emit_qkproj(hp, which, tq):
            # q^T / k^T projection for one head-pair over one token quarter
            w_sb = wq_sb if which == 0 else wk_sb
            dst = qft[hp] if which == 0 else kft[hp]
            b_sb = bq_sb if which == 0 else bk_sb
            p_ps = fps.tile([128, 512], F32, name="p_ps", tag="fill")
            for cc in range(NCC):
                nc.tensor.matmul(
                    p_ps,
                    lhsT=w_sb[cc][:, 128 * hp:128 * (hp + 1)],
                    rhs=xt[cc][:, 512 * tq:512 * (tq + 1)],
                    start=(cc == 0),
                    stop=(cc == NCC - 1),
                )
            nc.vector.tensor_scalar_add(
                dst[:, 512 * tq:512 * (tq + 1)], p_ps, b_sb[:, hp:hp + 1]
            )

        def emit_vproj(t):
            v_ps = fps.tile([128, F], F32, name="v_ps", tag="fill")
            for cc in range(NCC):
                nc.tensor.matmul(
                    v_ps,
                    lhsT=xt[cc][:, 128 * t:128 * (t + 1)],
                    rhs=wv_sb[cc],
                    start=(cc == 0),
                    stop=(cc == NCC - 1),
                )
            vv = v_aug[t].rearrange("p (h c) -> p h c", c=DA)
            nc.vector.tensor_copy(
                vv[:, :, 0:D], v_ps.rearrange("p (h c) -> p h c", c=D)
            )
            nc.vector.tensor_copy(
                vv[:, :, D:DA], ones_sb.rearrange("p (h o) -> p h o", o=1)
            )

        # y_norm / y^T tiles per block (double-buffered via tags)
        def make_yn(b):
            return [yp.tile([128, F], BF16, name=f"yn{b}_{s}", tag=f"yn{s}")
                    for s in range(4)]

        def make_yT(b):
            return [yp.tile([128, 512], BF16, name=f"yT{b}_{fc}", tag=f"yT{fc}")
                    for fc in range(NHP)]

        yn_cur = {}
        yT_cur = {}

        def emit_yT(b, s):
            # transpose y_norm[s] (q-major) into yT (feature-major)
            for fc in range(NHP):
                t_ps = fps.tile([128, 128], BF16, name="t_ps", tag="fill")
                nc.tensor.transpose(
                    t_ps, yn_cur[b][s][:, 128 * fc:128 * (fc + 1)], ident_sb
                )
                nc.vector.tensor_copy(
                    yT_cur[b][fc][:, 128 * s:128 * (s + 1)], t_ps
                )

        def emit_outproj(b, ti, eb):
            o_ps = fps.tile([128, 512], F32, name="o_ps", tag="fill")
            for fc in range(NHP):
                nc.tensor.matmul(
                    o_ps,
                    lhsT=yT_cur[b][fc][:, 128 * ti:128 * (ti + 1)],
                    rhs=wout_sb[fc][:, 512 * eb:512 * (eb + 1)],
                    start=(fc == 0),
                    stop=(fc == NHP - 1),
                )
            osb = osbp.tile([128, 512], F32, name="osb", tag="osb")
            nc.vector.tensor_copy(osb, o_ps)
            tt = 4 * b + ti
            nc.sync.dma_start(
                out=out_d[128 * tt:128 * (tt + 1), 512 * eb:512 * (eb + 1)],
                in_=osb,
            )

        # ---- prologue compute: first v tiles, then quarter-0 projections
        # (v inputs land first; wq/wk arrive while v-proj runs) ----
        for t in range(4):
            emit_vproj(t)
        for hp in range(NHP):
            emit_qkproj(hp, 0, 0)
            emit_qkproj(hp, 1, 0)

        # ---- filler closures per block ----
        fill = [[] for _ in range(NTB)]
        for b in range(NTB):
            if b + 1 < NTB:
                for hp in range(NHP):
                    fill[b].append(lambda hp=hp, tq=b + 1: emit_qkproj(hp, 0, tq))
                    fill[b].append(lambda hp=hp, tq=b + 1: emit_qkproj(hp, 1, tq))
                for t in range(4 * (b + 1), 4 * (b + 2)):
                    fill[b].append(lambda t=t: emit_vproj(t))
            if b >= 1:
                pb = b - 1
                for s in range(4):
                    fill[b].append(lambda pb=pb, s=s: emit_yT(pb, s))
                for ti in range(4):
                    for eb in range(2):
                        fill[b].append(
                            lambda pb=pb, ti=ti, eb=eb: emit_outproj(pb, ti, eb)
                        )

        # ---- attention ----
        pend = None
        for b in range(NTB):
            yn_cur[b] = make_yn(b)
            yT_cur[b] = make_yT(b)
            ng = 2 * b + 2
            slots = NHP * ng
            fl = fill[b]
            popped = 0
            slot = 0
            for hp in range(NHP):
                hA = 2 * hp
                hB = 2 * hp + 1
                yA = yps.tile([128, 4 * DA], F32, name="yA", tag="yA")
                yB = yps.tile([128, 4 * DA], F32, name="yB", tag="yB")
                eA, eB = {}, {}

                def pv_group(g, _b=b, _yA=yA, _yB=yB, _eA=eA, _eB=eB,
                             _hA=hA, _hB=hB):
                    # PV for both heads of score-group g (key tiles 2g, 2g+1),
                    # skipping tiles above the causal diagonal per q-slice
                    for h_half in range(2):
                        t = 2 * g + h_half
                        for y_ps, hh, e in ((_yA, _hA, _eA), (_yB, _hB, _eB)):
                            for s in range(4):
                                if t > 4 * _b + s:
                                    continue
                                # start only once per bank: CoreSim's
                                # start=True marks the whole 2KB zero region
                                # pending-zero, so later slices' first writes
                                # get their zero-fill from slice 0's start
                                nc.tensor.matmul(
                                    y_ps[:, DA * s:DA * s + DA],
                                    lhsT=e[g][:, 512 * h_half + 128 * s:
                                              512 * h_half + 128 * (s + 1)],
                                    rhs=v_aug[t][:, DA * hh:DA * (hh + 1)],
                                    start=(t == 0 and s == 0),
                                    stop=(t == 4 * _b + s),
                                    skip_group_check=True,
                                )

                def finish_hp(_b=b, _hp=hp, _ng=ng, _pv=pv_group,
                              _yA=yA, _yB=yB, _hA=hA, _hB=hB):
                    # last group's PV, then fold normalization into the
                    # PSUM->SBUF copies (per-partition reciprocal scalars)
                    _pv(_ng - 1)
                    rc = smallp.tile([128, 8], F32, name="rc", tag="rc")
                    rc3 = rc.rearrange("p (s o) -> p s o", o=1)
                    yAr = _yA.rearrange("p (s c) -> p s c", c=DA)
                    yBr = _yB.rearrange("p (s c) -> p s c", c=DA)
                    nc.vector.reciprocal(rc3[:, 0:4, :], yAr[:, :, D:DA])
                    nc.vector.reciprocal(rc3[:, 4:8, :], yBr[:, :, D:DA])
                    for s in range(4):
                        nc.vector.tensor_scalar_mul(
                            yn_cur[_b][s][:, D * _hA:D * (_hA + 1)],
                            _yA[:, DA * s:DA * s + D],
                            rc[:, s:s + 1],
                        )
                        nc.vector.tensor_scalar_mul(
                            yn_cur[_b][s][:, D * _hB:D * (_hB + 1)],
                            _yB[:, DA * s:DA * s + D],
                            rc[:, 4 + s:5 + s],
                        )

                for g in range(ng):
                    # scores for both heads; diagonal tiles are F-trimmed
                    sA = sps.tile([128, 1024], F32, name="sA", tag="sA")
                    sB = sps.tile([128, 1024], F32, name="sB", tag="sB")
                    offs = []
                    for h_half in range(2):
                        t = 2 * g + h_half
                        off = 128 * (t - 4 * b) if t >= 4 * b else 0
                        offs.append(off)
                        for s_ps, po in ((sA, 0), (sB, 64)):
                            nc.tensor.matmul(
                                s_ps[:, 512 * h_half + off:512 * (h_half + 1)],
                                lhsT=kft[hp][po:po + 64, 128 * t:128 * (t + 1)],
                                rhs=qft[hp][po:po + 64,
                                            512 * b + off:512 * (b + 1)],
                                start=True,
                                stop=True,
                            )
                    eA[g] = ep.tile([128, 1024], BF16, name="eA", tag="eA")
                    eB[g] = ep.tile([128, 1024], BF16, name="eB", tag="eB")
                    for e_t, s_ps in ((eA[g], sA), (eB[g], sB)):
                        if offs[0] == 0 and offs[1] == 0:
                            nc.scalar.activation(e_t, s_ps, AF.Exp, scale=SCALE)
                        else:
                            for h_half in range(2):
                                lo = 512 * h_half + offs[h_half]
                                hi = 512 * (h_half + 1)
                                nc.scalar.activation(
                                    e_t[:, lo:hi], s_ps[:, lo:hi],
                                    AF.Exp, scale=SCALE,
                                )
                    # triangular masks on the diagonal 128x128 blocks
                    for h_half in range(2):
                        t = 2 * g + h_half
                        j = t - 4 * b
                        if 0 <= j < 4:
                            lo = 512 * h_half + 128 * j
                            nc.gpsimd.tensor_mul(
                                eA[g][:, lo:lo + 128], eA[g][:, lo:lo + 128],
                                tri_sb,
                            )
                            nc.gpsimd.tensor_mul(
                                eB[g][:, lo:lo + 128], eB[g][:, lo:lo + 128],
                                tri_sb,
                            )
                    if _DEBUG and b == 0 and hp == 0 and g == 0:
                        nc.sync.dma_start(out=dbg_e[:, 0:512],
                                          in_=eA[g][:, 0:512])
                        nc.sync.dma_start(out=dbg_e[:, 640:1024],
                                          in_=eA[g][:, 640:1024])
                    if _DEBUG and b == 1 and hp == 0 and g == 1:
                        nc.sync.dma_start(out=dbg_yn[:, :], in_=yn_cur[0][0])
                    if g == 0 and pend is not None:
                        pend()
                        pend = None
                    if g >= 1:
                        pv_group(g - 1)
                    # drain filler evenly across this block's slots
                    slot += 1
                    target = math.ceil(len(fl) * slot / slots)
                    while popped < target:
                        fl[popped]()
                        popped += 1
                pend = finish_hp
        pend()

        if _DEBUG:
            nc.sync.dma_start(out=dbg_qft[:, :], in_=qft[0])
            nc.sync.dma_start(out=dbg_kft[:, :], in_=kft[0])
            nc.sync.dma_start(out=dbg_v[:, :], in_=v_aug[0])

        # ---- epilogue: last block's transpose + out-projection ----
        for s in range(4):
            emit_yT(NTB - 1, s)
        for ti in range(4):
            for eb in range(2):
                emit_outproj(NTB - 1, ti, eb)

    nc.finalize()
    return nc


def _host_constants():
    import ml_dtypes
    bf16 = ml_dtypes.bfloat16
    ident = np.eye(128, dtype=np.float32).astype(bf16)
    jj = np.arange(128)[:, None]
    ii = np.arange(128)[None, :]
    tri = (jj <= ii).astype(np.float32).astype(bf16)
    return ident, tri


def _in_maps(x, w_qkv, b_qkv, w_out):
    import ml_dtypes
    bf16 = ml_dtypes.bfloat16
    ident, tri = _host_constants()
    maps = []
    for core in range(8):
        b, g = core // 2, core % 2
        cols = slice(512 * g, 512 * (g + 1))
        wq = np.ascontiguousarray(w_qkv[:, 0 * C:1 * C][:, cols]).astype(bf16)
        wk = np.ascontiguousarray(w_qkv[:, 1 * C:2 * C][:, cols]).astype(bf16)
        wv = np.ascontiguousarray(w_qkv[:, 2 * C:3 * C][:, cols]).astype(bf16)
        bq = np.ascontiguousarray(
            b_qkv[0 * C:1 * C][cols].reshape(NHP, 128).T
        ).astype(np.float32)
        bk = np.ascontiguousarray(
            b_qkv[1 * C:2 * C][cols].reshape(NHP, 128).T
        ).astype(np.float32)
        wo = np.ascontiguousarray(w_out[cols, :]).astype(bf16)
        xt_full = np.ascontiguousarray(x[b].T).astype(bf16)
        maps.append(
            {
                "xt": xt_full,
                "wq": wq,
                "wk": wk,
                "wv": wv,
                "wout": wo,
                "bq": bq,
                "bk": bk,
                "tri": tri,
                "ident": ident,
            }
        )
    return maps


_NC_CACHE = {}


def _get_nc():
    if "nc" not in _NC_CACHE:
        _NC_CACHE["nc"] = build_nc()
    return _NC_CACHE["nc"]


def run(inputs, trace=False, **spmd_kwargs):
    """Returns (output, BassKernelResults)."""
    x = np.asarray(inputs["x"], dtype=np.float32)
    w_qkv = np.asarray(inputs["w_qkv"], dtype=np.float32)
    b_qkv = np.asarray(inputs["b_qkv"], dtype=np.float32)
    w_out = np.asarray(inputs["w_out"], dtype=np.float32)
    b_out = np.asarray(inputs["b_out"], dtype=np.float32)

    nc = _get_nc()
    maps = _in_maps(x, w_qkv, b_qkv, w_out)
    res = run_bass_kernel_spmd(
        nc, maps, list(range(8)), trace=trace, **spmd_kwargs
    )
    out = np.empty((B, T, C), dtype=np.float32)
    for b in range(B):
        out[b] = res.results[2 * b]["out"] + res.results[2 * b + 1]["out"]
    # softmax rows sum to 1, so the v-bias passes through attention unchanged:
    # its contribution to the output is b_v @ w_out, added once on the host.
    bias = b_out + b_qkv[2 * C:3 * C] @ w_out
    out += bias[None, None, :]
    return out, res


def kernel(**inputs):
    out, _ = run(inputs, trace=False)
    return out
